# revision 1
# baseline (speedup 1.0000x reference)
"""LucidLinearAttention Trainium2 kernel (8-core SPMD).

Sharding: batch b = core//2 (4 batches), head-group hg = core%2 (8 heads each).
Each core computes qkv projection for its heads, chunked linear attention
(bucket-exclusive cumsum) via a hybrid block-causal formulation, and its
partial output projection. Host sums the two head-group partials per batch.

All matmul accumulation groups use lhsT/rhs at partition base 0 with uniform
K (mixed-base accumulation groups crash the HW - validated by bisection).
"""
import sys
import numpy as np

for p in ("/opt/trn_rl_repo", "/root/.axon_site/_ro/trn_rl_repo"):
    if p not in sys.path:
        sys.path.insert(0, p)

import concourse.mybir as mybir
import concourse.tile as tile
from concourse import bacc
from concourse.bass_utils import run_bass_kernel_spmd
from concourse.masks import make_identity

F32 = mybir.dt.float32
F32R = mybir.dt.float32r
EXP = mybir.ActivationFunctionType.Exp

B, T, D = 4, 4096, 1024
NH, HD, BUCKET = 16, 64, 64
HPC = 8            # heads per core
GD = HPC * HD      # 512 group dim
NBLK = 8           # coarse blocks
BT = T // NBLK     # 512 rows per block
NC_CORES = 8

_CACHE = {}


def _build():
    nc = bacc.Bacc("TRN2", target_bir_lowering=False, debug=False,
                   num_devices=NC_CORES)
    xT = nc.dram_tensor("xT", [D, T], F32, kind="ExternalInput").ap()
    wqT = nc.dram_tensor("wqT", [D, GD], F32, kind="ExternalInput").ap()
    wkT = nc.dram_tensor("wkT", [D, GD], F32, kind="ExternalInput").ap()
    wvT = nc.dram_tensor("wvT", [D, GD], F32, kind="ExternalInput").ap()
    woT = nc.dram_tensor("woT", [GD, D], F32, kind="ExternalInput").ap()
    y = nc.dram_tensor("y", [T, D], F32, kind="ExternalOutput").ap()

    with tile.TileContext(nc) as tc:
        with nc.allow_low_precision(reason="float32r matmul rounding by design"), \
             tc.tile_pool(name="w", bufs=1) as wp, \
             tc.tile_pool(name="per", bufs=1) as pp, \
             tc.tile_pool(name="sb", bufs=1) as sbp, \
             tc.tile_pool(name="ps", bufs=1, space="PSUM") as ps:

            # ---- resident weights -------------------------------------
            wq_sb = [wp.tile([128, GD], F32R, tag=f"wq{dc}", name=f"wq{dc}") for dc in range(8)]
            wk_sb = [wp.tile([128, GD], F32R, tag=f"wk{dc}", name=f"wk{dc}") for dc in range(8)]
            wv_sb = [wp.tile([128, GD], F32R, tag=f"wv{dc}", name=f"wv{dc}") for dc in range(8)]
            wo_sb = [wp.tile([64, D], F32R, tag=f"wo{h}", name=f"wo{h}") for h in range(HPC)]
            for dc in range(8):
                for src_ap, dst in ((wqT, wq_sb), (wkT, wk_sb), (wvT, wv_sb)):
                    stg = sbp.tile([128, GD], F32, tag="stage", name="stage", bufs=2)
                    nc.sync.dma_start(stg[:], src_ap[128 * dc:128 * (dc + 1), :])
                    nc.vector.tensor_copy(dst[dc][:], stg[:])
            for h in range(HPC):
                stg = sbp.tile([64, D], F32, tag="wstage", name="wstage", bufs=2)
                nc.sync.dma_start(stg[:], woT[64 * h:64 * (h + 1), :])
                nc.vector.tensor_copy(wo_sb[h][:], stg[:])

            # ---- persistent state -------------------------------------
            ident = pp.tile([128, 128], F32, tag="ident")
            make_identity(nc, ident[:])
            ident_r = pp.tile([128, 128], F32R, tag="ident_r")
            nc.vector.tensor_copy(ident_r[:], ident[:])
            # F32 staging constants (memset on F32R is invalid ISA; fp32r
            # tiles must be produced by rounding compute instructions).
            zero_f32 = pp.tile([128, BT], F32, tag="zero_f32")
            nc.vector.memset(zero_f32[:], 0.0)
            one_f32 = pp.tile([128, 16], F32, tag="one_f32")
            nc.vector.memset(one_f32[:], 1.0)
            # bvec: K=2 broadcast weights; row 64 = 1, row 65 = 0.
            bv_f32 = pp.tile([66, 64], F32, tag="bv_f32")
            nc.vector.memset(bv_f32[:], 0.0)
            nc.vector.memset(bv_f32[64:65, :], 1.0)
            bvec = pp.tile([66, 64], F32R, tag="bvec")
            nc.vector.tensor_copy(bvec[:], bv_f32[:])
            # qtu_h: [128, BT]; rows 0-63 = exp(q) of head h (d x t),
            # rows 64-127 permanently zero (K=128 inter matmul padding).
            qtu = [pp.tile([128, BT], F32R, tag=f"qtu{h}", name=f"qtu{h}") for h in range(HPC)]
            for h in range(HPC):
                nc.vector.tensor_copy(qtu[h][:], zero_f32[:])
            # caug_h: [128, 66]; rows 0-63 = [C (d x e) | kcum | pad], rest 0.
            caug = [pp.tile([128, 66], F32R, tag=f"caug{h}", name=f"caug{h}") for h in range(HPC)]
            for h in range(HPC):
                nc.vector.tensor_copy(caug[h][:], zero_f32[:, 0:66])
            # vaug[tc]: [128, 8*66]; per head h cols h*66..h*66+64 = V,
            # col h*66+64 = ones (den trick), col h*66+65 = zero pad.
            vaug = [pp.tile([128, HPC * 66], F32R, tag=f"vaug{t}", name=f"vaug{t}") for t in range(4)]
            one_col = one_f32[:].rearrange("p (a b) -> p a b", b=1)[:, 0:8, :]
            zero_col = zero_f32[:, 0:8].rearrange("p (a b) -> p a b", b=1)
            for t4 in range(4):
                vv = vaug[t4][:].rearrange("p (h c) -> p h c", c=66)
                nc.vector.tensor_copy(vv[:, :, 64:65], one_col)
                nc.vector.tensor_copy(vv[:, :, 65:66], zero_col)
            # ssb: 2 parity sets x 4 chunks of masked S^T [128, BT].
            # Zero strips are preset once and never overwritten.
            ssb = [[pp.tile([128, BT], F32R, tag=f"ssb{s}_{t}", name=f"ssb{s}_{t}") for t in range(4)]
                   for s in range(2)]
            for s in range(2):
                for t4 in range(4):
                    nc.vector.tensor_copy(ssb[s][t4][:], zero_f32[:])

            # ---- main loop over coarse blocks -------------------------
            for ct in range(NBLK):
                t0 = ct * BT
                # x^T tiles for this block: [d-chunk 128, t 512]
                xsb = [sbp.tile([128, BT], F32R, tag=f"xsb{dc}", name=f"xsb{dc}") for dc in range(8)]
                for dc in range(8):
                    xstg = sbp.tile([128, BT], F32, tag="xstage", name="xstage", bufs=2)
                    nc.sync.dma_start(
                        xstg[:], xT[128 * dc:128 * (dc + 1), t0:t0 + BT])
                    nc.vector.tensor_copy(xsb[dc][:], xstg[:])

                # Q^T projection per head (M=64) + exp
                for h in range(HPC):
                    pq = ps.tile([64, BT], F32, tag="big")
                    for dc in range(8):
                        nc.tensor.matmul(
                            pq[:], wq_sb[dc][:, 64 * h:64 * (h + 1)], xsb[dc][:],
                            start=(dc == 0), stop=(dc == 7))
                    nc.scalar.activation(qtu[h][0:64, :], pq[:], EXP)

                # K natural projection per t-chunk (M=128) + exp
                ksb = [sbp.tile([128, GD], F32R, tag=f"ksb{t}", name=f"ksb{t}") for t in range(4)]
                for t4 in range(4):
                    pk = ps.tile([128, GD], F32, tag="big")
                    for dc in range(8):
                        nc.tensor.matmul(
                            pk[:], xsb[dc][:, 128 * t4:128 * (t4 + 1)], wk_sb[dc][:],
                            start=(dc == 0), stop=(dc == 7))
                    nc.scalar.activation(ksb[t4][:], pk[:], EXP)

                # V projection per t-chunk -> vaug strided cols
                for t4 in range(4):
                    pv = ps.tile([128, GD], F32, tag="big")
                    for dc in range(8):
                        nc.tensor.matmul(
                            pv[:], xsb[dc][:, 128 * t4:128 * (t4 + 1)], wv_sb[dc][:],
                            start=(dc == 0), stop=(dc == 7))
                    vv = vaug[t4][:].rearrange("p (h c) -> p h c", c=66)
                    pvv = pv[:].rearrange("p (h c) -> p h c", c=64)
                    nc.vector.tensor_copy(vv[:, :, 0:64], pvv[:, :, :])

                # K^T per head via PE transpose: kt_h [64, BT]
                kt = [sbp.tile([64, BT], F32R, tag=f"kt{h}", name=f"kt{h}") for h in range(HPC)]
                for h in range(HPC):
                    for t4 in range(4):
                        pt = ps.tile([64, 128], F32R, tag="small")
                        nc.tensor.transpose(
                            pt[:], ksb[t4][:, 64 * h:64 * (h + 1)], ident_r[:])
                        nc.vector.tensor_copy(
                            kt[h][:, 128 * t4:128 * (t4 + 1)], pt[:])

                # ---- attention per head -------------------------------
                xots = []
                for h in range(HPC):
                    par = h % 2
                    # S^T chunks + masked region copies
                    for t4 in range(4):
                        pst = ps.tile([128, BT], F32, tag="s")
                        nc.tensor.matmul(
                            pst[:], kt[h][:, 128 * t4:128 * (t4 + 1)],
                            qtu[h][0:64, :], start=True, stop=True)
                        c0 = (2 * t4 + 1) * 64
                        c1 = (2 * t4 + 2) * 64
                        nc.scalar.copy(ssb[par][t4][0:64, c0:BT], pst[0:64, c0:BT])
                        if c1 < BT:
                            nc.scalar.copy(
                                ssb[par][t4][64:128, c1:BT], pst[64:128, c1:BT])

                    # OUT group: inter (K=128, zero-padded) + 4 intra partial-N
                    po = ps.tile([66, BT], F32, tag="o")
                    nc.tensor.matmul(po[:], caug[h][:, :], qtu[h][:, :],
                                     start=True, stop=False)
                    for t4 in range(4):
                        n0 = (2 * t4 + 1) * 64
                        nc.tensor.matmul(
                            po[0:66, n0:BT],
                            vaug[t4][:, 66 * h:66 * h + 66],
                            ssb[par][t4][:, n0:BT],
                            start=False, stop=(t4 == 3))

                    # normalize: dinv = 1/max(den,eps); bcast via K=1 matmul
                    dv = sbp.tile([66, BT], F32R, tag="dv")
                    nc.vector.tensor_scalar_max(dv[64:66, :], po[64:66, :], 1e-30)
                    nc.vector.reciprocal(dv[64:66, :], dv[64:66, :])
                    pb = ps.tile([64, BT], F32, tag="small")
                    nc.tensor.matmul(pb[:], bvec[64:66, 0:64], dv[64:66, :],
                                     start=True, stop=True)
                    sbb = sbp.tile([64, BT], F32, tag="sbb")
                    nc.scalar.copy(sbb[:], pb[:])
                    xot = sbp.tile([64, BT], F32R, tag=f"xot{h}")
                    nc.vector.tensor_mul(xot[:], po[0:64, :], sbb[:])

                    # C/kcum update (after inter read): caug += K^T @ V_aug
                    pc = ps.tile([64, 66], F32, tag="small")
                    for t4 in range(4):
                        nc.tensor.matmul(
                            pc[:], ksb[t4][:, 64 * h:64 * (h + 1)],
                            vaug[t4][:, 66 * h:66 * h + 66],
                            start=(t4 == 0), stop=(t4 == 3))
                    nc.vector.tensor_add(caug[h][0:64, :], caug[h][0:64, :], pc[:])

                    xots.append(xot)

                # partial output projection + store y block
                for t4 in range(4):
                    for fc in range(2):
                        py = ps.tile([128, GD], F32, tag="big")
                        for h in range(HPC):
                            nc.tensor.matmul(
                                py[:],
                                xots[h][:, 128 * t4:128 * (t4 + 1)],
                                wo_sb[h][:, GD * fc:GD * (fc + 1)],
                                start=(h == 0), stop=(h == HPC - 1))
                        ysb = sbp.tile([128, GD], F32, tag="ysb")
                        nc.vector.tensor_copy(ysb[:], py[:])
                        nc.sync.dma_start(
                            y[t0 + 128 * t4:t0 + 128 * (t4 + 1),
                              GD * fc:GD * (fc + 1)], ysb[:])

    nc.compile()
    return nc


def _get_nc():
    if "nc" not in _CACHE:
        _CACHE["nc"] = _build()
    return _CACHE["nc"]


def kernel(x, W_qkv, W_out):
    x = np.asarray(x, dtype=np.float32)
    W_qkv = np.asarray(W_qkv, dtype=np.float32)
    W_out = np.asarray(W_out, dtype=np.float32)
    nc = _get_nc()

    xTs = [np.ascontiguousarray(x[b].T) for b in range(B)]
    in_maps = []
    for c in range(NC_CORES):
        b, hg = c // 2, c % 2
        s = slice(hg * GD, (hg + 1) * GD)
        in_maps.append({
            "xT": xTs[b],
            "wqT": np.ascontiguousarray(W_qkv[0 * D:1 * D][s].T),
            "wkT": np.ascontiguousarray(W_qkv[1 * D:2 * D][s].T),
            "wvT": np.ascontiguousarray(W_qkv[2 * D:3 * D][s].T),
            "woT": np.ascontiguousarray(W_out[:, s].T),
        })
    res = run_bass_kernel_spmd(nc, in_maps, core_ids=list(range(NC_CORES)))
    out = np.empty((B, T, D), dtype=np.float32)
    for b in range(B):
        out[b] = res.results[2 * b]["y"] + res.results[2 * b + 1]["y"]
    return out



# revision 7
# speedup vs baseline: 2.7099x; 2.7099x over previous
"""LucidLinearAttention Trainium2 kernel (8-core SPMD), bf16 dataflow.

Sharding: batch b = core//2 (4 batches), head-group hg = core%2 (8 heads each).
Each core: qkv projection for its heads, chunked linear attention with a
hybrid block-causal formulation (256-col attention blocks, 64-wide buckets,
exclusive block carry C in f32), partial output projection. Host sums the two
head-group partials per batch.

Head-pair (h2) stacking keeps every projection / out-projection matmul at
M=K=128. S^T is computed only over the causally visible column range; the
bucket mask is realized by copying rect+wedge regions into pre-zeroed SBUF
tiles. All matmul operands are bf16 (predicted rel err ~6e-3 vs 2e-2 gate);
PSUM accumulation stays f32.
"""
import sys
import numpy as np

for p in ("/opt/trn_rl_repo", "/root/.axon_site/_ro/trn_rl_repo"):
    if p not in sys.path:
        sys.path.insert(0, p)

import ml_dtypes
import concourse.mybir as mybir
import concourse.tile as tile
from concourse import bacc
from concourse.bass_utils import run_bass_kernel_spmd
from concourse.masks import make_identity

F32 = mybir.dt.float32
F32R = mybir.dt.float32r
BF16 = mybir.dt.bfloat16
EXP = mybir.ActivationFunctionType.Exp

B, T, D = 4, 4096, 1024
NH, HD, BUCKET = 16, 64, 64
HPC = 8                 # heads per core
GD = HPC * HD           # 512
NPB = 8                 # projection blocks
PBT = T // NPB          # 512 cols
NAB = 16                # attention blocks
ABT = T // NAB          # 256 cols
NC_CORES = 8

_CACHE = {}
_BF = ml_dtypes.bfloat16


def _build():
    nc = bacc.Bacc("TRN2", target_bir_lowering=False, debug=False,
                   num_devices=NC_CORES)
    xT = nc.dram_tensor("xT", [D, T], BF16, kind="ExternalInput").ap()
    wqT = nc.dram_tensor("wqT", [D, GD], BF16, kind="ExternalInput").ap()
    wkT = nc.dram_tensor("wkT", [D, GD], BF16, kind="ExternalInput").ap()
    wvT = nc.dram_tensor("wvT", [D, GD], BF16, kind="ExternalInput").ap()
    woT = nc.dram_tensor("woT", [GD, D], BF16, kind="ExternalInput").ap()
    y = nc.dram_tensor("y", [T, D], F32, kind="ExternalOutput").ap()

    with tile.TileContext(nc) as tc:
        with nc.allow_low_precision(reason="bf16 matmul dataflow by design"), \
             tc.tile_pool(name="w", bufs=1) as wp, \
             tc.tile_pool(name="per", bufs=1) as pp, \
             tc.tile_pool(name="sb", bufs=1) as sbp, \
             tc.tile_pool(name="ps", bufs=1, space="PSUM") as ps:

            # ---- resident weights (direct bf16 DMA) ---------------------
            wq_sb = [wp.tile([128, GD], BF16, tag=f"wq{dc}", name=f"wq{dc}") for dc in range(8)]
            wk_sb = [wp.tile([128, GD], BF16, tag=f"wk{dc}", name=f"wk{dc}") for dc in range(8)]
            wv_sb = [wp.tile([128, GD], BF16, tag=f"wv{dc}", name=f"wv{dc}") for dc in range(8)]
            wo_sb = [wp.tile([128, D], BF16, tag=f"wo{h2}", name=f"wo{h2}") for h2 in range(4)]

            def load_x(pb0):
                xs = [sbp.tile([128, 2 * PBT], BF16, tag=f"xsb{dc}", name=f"xsb{dc}", bufs=2)
                      for dc in range(8)]
                for dc in range(8):
                    nc.sync.dma_start(
                        xs[dc][:], xT[128 * dc:128 * (dc + 1), PBT * pb0:PBT * (pb0 + 2)])
                return xs

            for dc in range(8):
                nc.sync.dma_start(wq_sb[dc][:], wqT[128 * dc:128 * (dc + 1), :])
            xsb = load_x(0)
            for dc in range(8):
                nc.gpsimd.dma_start(wk_sb[dc][:], wkT[128 * dc:128 * (dc + 1), :])
            for dc in range(8):
                nc.gpsimd.dma_start(wv_sb[dc][:], wvT[128 * dc:128 * (dc + 1), :])
            for h2 in range(4):
                nc.gpsimd.dma_start(wo_sb[h2][:], woT[128 * h2:128 * (h2 + 1), :])
            xsb_next = load_x(2)

            # ---- persistent state --------------------------------------
            ident_f = pp.tile([128, 128], F32, tag="ident_f")
            make_identity(nc, ident_f[:])
            ident_b = pp.tile([128, 128], BF16, tag="ident_b")
            nc.vector.tensor_copy(ident_b[:], ident_f[:])
            one_f = pp.tile([1, 64], F32, tag="one_f")
            nc.vector.memset(one_f[:], 1.0)
            ones_r = pp.tile([1, 64], F32R, tag="ones_r")
            nc.vector.tensor_copy(ones_r[:], one_f[:])
            # caug[h]: [128, 66] bf16; even h data in rows 0:64, odd in 64:128,
            # other half stays zero (K=128 padding for the inter matmul).
            caug = [pp.tile([128, 66], BF16, tag=f"caug{h}", name=f"caug{h}") for h in range(HPC)]
            for h in range(HPC):
                nc.vector.memset(caug[h][:], 0.0)
            caug_f32 = [pp.tile([64, 66], F32, tag=f"caugf{h}", name=f"caugf{h}") for h in range(HPC)]
            for h in range(HPC):
                nc.vector.memset(caug_f32[h][:], 0.0)
            # vaug[par][c]: [128, 8*66] bf16; per head slot: [v(64) | 1 | 0]
            vaug = [[pp.tile([128, HPC * 66], BF16, tag=f"vaug{par}_{c}", name=f"vaug{par}_{c}")
                     for c in range(4)] for par in range(2)]
            for par in range(2):
                for c in range(4):
                    nc.vector.memset(vaug[par][c][:], 0.0)
                    for h in range(HPC):
                        nc.vector.memset(vaug[par][c][:, 66 * h + 64:66 * h + 65], 1.0)
            # ssb[h][par]: [128, 320] bf16 masked S^T; cols 0:256 chunk0
            # (queries 0:256), cols 256:320 chunk1 wedge (queries 192:256).
            # Zero strips preset once.
            ssb = [[pp.tile([128, 320], BF16, tag=f"ssb{h}_{s}", name=f"ssb{h}_{s}")
                    for s in range(2)] for h in range(HPC)]
            for h in range(HPC):
                for s in range(2):
                    nc.vector.memset(ssb[h][s][:], 0.0)

            pending = []  # out-projection work deferred one attn block

            def emit_outproj(xot2_p, a_p):
                qoff_p = ABT * (a_p % 2)
                for tch in range(2):
                    r0 = ABT * a_p + 128 * tch
                    ysb = sbp.tile([128, D], F32, tag="ysb", name="ysb", bufs=2)
                    for fc in range(2):
                        py = ps.tile([128, GD], F32, tag="proj", name="py", bufs=2)
                        for h2 in range(4):
                            nc.tensor.matmul(
                                py[:], xot2_p[h2][:, qoff_p + 128 * tch:qoff_p + 128 * (tch + 1)],
                                wo_sb[h2][:, GD * fc:GD * (fc + 1)],
                                start=(h2 == 0), stop=(h2 == 3))
                        nc.scalar.copy(ysb[:, GD * fc:GD * (fc + 1)], py[:])
                    nc.sync.dma_start(y[r0:r0 + 128, :], ysb[:])

            # ---- main loop over projection blocks ----------------------
            for pb in range(NPB):
                par = pb % 2
                xoff = PBT * par  # column base inside the [128,1024] x tiles
                if pb >= 2 and par == 0:
                    xsb = xsb_next
                    if pb + 2 < NPB:
                        xsb_next = load_x(pb + 2)

                # Q^T projection, head pairs stacked (M=128) + exp -> bf16
                qtu2 = []
                for h2 in range(4):
                    pq = ps.tile([128, PBT], F32, tag="proj", name="pq", bufs=2)
                    for dc in range(8):
                        nc.tensor.matmul(
                            pq[:], wq_sb[dc][:, 128 * h2:128 * (h2 + 1)],
                            xsb[dc][:, xoff:xoff + PBT],
                            start=(dc == 0), stop=(dc == 7))
                    qt = sbp.tile([128, PBT], BF16, tag=f"qtu{h2}", name=f"qtu{h2}", bufs=2)
                    nc.scalar.activation(qt[:], pq[:], EXP)
                    qtu2.append(qt)

                # K natural projection per 128-t chunk + exp -> bf16
                ksb = []
                for c in range(4):
                    pk = ps.tile([128, GD], F32, tag="proj", name="pk", bufs=2)
                    for dc in range(8):
                        nc.tensor.matmul(
                            pk[:], xsb[dc][:, xoff + 128 * c:xoff + 128 * (c + 1)],
                            wk_sb[dc][:],
                            start=(dc == 0), stop=(dc == 7))
                    kt_ = sbp.tile([128, GD], BF16, tag=f"ksb{c}", name=f"ksb{c}", bufs=2)
                    nc.scalar.activation(kt_[:], pk[:], EXP)
                    ksb.append(kt_)

                # V projection per chunk -> vaug strided slots (bf16)
                for c in range(4):
                    pv = ps.tile([128, GD], F32, tag="proj", name="pv", bufs=2)
                    for dc in range(8):
                        nc.tensor.matmul(
                            pv[:], xsb[dc][:, xoff + 128 * c:xoff + 128 * (c + 1)],
                            wv_sb[dc][:],
                            start=(dc == 0), stop=(dc == 7))
                    vv = vaug[par][c][:].rearrange("p (h c) -> p h c", c=66)
                    pvv = pv[:].rearrange("p (h c) -> p h c", c=64)
                    nc.vector.tensor_copy(vv[:, :, 0:64], pvv[:, :, :])

                # K^T via PE transpose, head pairs stacked (bf16 psum)
                kt2 = []
                for h2 in range(4):
                    ktp = ps.tile([128, PBT], BF16, tag="spb", name="ktp", bufs=3)
                    for c in range(4):
                        nc.tensor.transpose(
                            ktp[:, 128 * c:128 * (c + 1)],
                            ksb[c][:, 128 * h2:128 * (h2 + 1)], ident_b[:])
                    kt = sbp.tile([128, PBT], BF16, tag=f"kt2{h2}", name=f"kt2{h2}", bufs=2)
                    nc.vector.tensor_copy(kt[:], ktp[:])
                    kt2.append(kt)

                xot2 = [sbp.tile([128, PBT], BF16, tag=f"xot{h2}", name=f"xot{h2}", bufs=2)
                        for h2 in range(4)]

                # ---- attention blocks (2 per proj block) ----------------
                for ab in range(2):
                    a = 2 * pb + ab
                    qoff = ABT * ab
                    spar = a % 2

                    # S^T (visible range only) + masked rect/wedge copies
                    psts = []
                    for h in range(HPC):
                        h2, hb = h // 2, (h % 2) * 64
                        pst = ps.tile([128, ABT], F32, tag="spb", name="pst", bufs=3)
                        nc.tensor.matmul(
                            pst[:, 0:192],
                            kt2[h2][hb:hb + 64, qoff:qoff + 128],
                            qtu2[h2][hb:hb + 64, qoff + 64:qoff + 256],
                            start=True, stop=True)
                        nc.tensor.matmul(
                            pst[:, 192:256],
                            kt2[h2][hb:hb + 64, qoff + 128:qoff + 256],
                            qtu2[h2][hb:hb + 64, qoff + 192:qoff + 256],
                            start=True, stop=True)
                        sb_t = ssb[h][spar]
                        nc.vector.tensor_copy(sb_t[:, 128:256], pst[:, 64:192])
                        nc.scalar.copy(sb_t[0:64, 64:128], pst[0:64, 0:64])
                        nc.scalar.copy(sb_t[0:64, 256:320], pst[0:64, 192:256])
                        psts.append(pst)

                    # previous attn block's out-projection (PE cover for copies)
                    if pending:
                        emit_outproj(*pending.pop())

                    # OUT groups + normalization, software-pipelined by 2 heads
                    opbs, dvs, xus = [None] * HPC, [None] * HPC, [None] * HPC

                    def norm_tail(hh):
                        h2h, hbh = hh // 2, (hh % 2) * 64
                        nc.tensor.matmul(opbs[hh][0:64, 256:512], ones_r[:, :],
                                         dvs[hh][:], start=True, stop=True)
                        xus[hh] = sbp.tile([64, ABT], BF16, tag="xotu", name="xotu", bufs=3)
                        nc.scalar.copy(xus[hh][:], opbs[hh][0:64, 0:256])
                        nc.vector.tensor_mul(
                            xot2[h2h][hbh:hbh + 64, qoff:qoff + 256],
                            opbs[hh][0:64, 256:512], xus[hh][:])

                    for h in range(HPC):
                        h2, hb = h // 2, (h % 2) * 64
                        opb = ps.tile([128, 512], F32, tag="opb", name="opb", bufs=3)
                        opbs[h] = opb
                        nc.tensor.matmul(opb[0:66, 0:256], caug[h][:],
                                         qtu2[h2][:, qoff:qoff + 256],
                                         start=True, stop=False)
                        nc.tensor.matmul(opb[0:66, 64:256],
                                         vaug[par][2 * ab][:, 66 * h:66 * h + 66],
                                         ssb[h][spar][:, 64:256],
                                         start=False, stop=False)
                        nc.tensor.matmul(opb[0:66, 192:256],
                                         vaug[par][2 * ab + 1][:, 66 * h:66 * h + 66],
                                         ssb[h][spar][:, 256:320],
                                         start=False, stop=True)
                        dv = sbp.tile([1, ABT], F32R, tag="dv", name="dv", bufs=4)
                        if a == 0:
                            nc.vector.tensor_scalar_max(dv[:], opb[64:65, 0:256], 1e-30)
                            nc.vector.reciprocal(dv[:], dv[:])
                        else:
                            nc.vector.reciprocal(dv[:], opb[64:65, 0:256])
                        dvs[h] = dv
                        # C / kcum update for this head (inter read is done)
                        pc = ps.tile([64, 66], F32, tag="spb", name="pc", bufs=3)
                        for ci in range(2):
                            c = 2 * ab + ci
                            nc.tensor.matmul(
                                pc[:], ksb[c][:, 64 * h:64 * (h + 1)],
                                vaug[par][c][:, 66 * h:66 * h + 66],
                                start=(ci == 0), stop=(ci == 1))
                        nc.vector.tensor_add(caug_f32[h][:], caug_f32[h][:], pc[:])
                        nc.gpsimd.tensor_copy(caug[h][hb:hb + 64, :], caug_f32[h][:])
                        if h >= 2:
                            norm_tail(h - 2)
                    norm_tail(6)
                    norm_tail(7)

                    pending.append((xot2, a))

            emit_outproj(*pending.pop())

    nc.compile()
    return nc


def _get_nc():
    if "nc" not in _CACHE:
        _CACHE["nc"] = _build()
    return _CACHE["nc"]


def kernel(x, W_qkv, W_out):
    x = np.asarray(x, dtype=np.float32)
    W_qkv = np.asarray(W_qkv, dtype=np.float32)
    W_out = np.asarray(W_out, dtype=np.float32)
    nc = _get_nc()

    xTs = [np.ascontiguousarray(x[b].T).astype(_BF) for b in range(B)]
    in_maps = []
    for c in range(NC_CORES):
        b, hg = c // 2, c % 2
        s = slice(hg * GD, (hg + 1) * GD)
        in_maps.append({
            "xT": xTs[b],
            "wqT": np.ascontiguousarray(W_qkv[0 * D:1 * D][s].T).astype(_BF),
            "wkT": np.ascontiguousarray(W_qkv[1 * D:2 * D][s].T).astype(_BF),
            "wvT": np.ascontiguousarray(W_qkv[2 * D:3 * D][s].T).astype(_BF),
            "woT": np.ascontiguousarray(W_out[:, s].T).astype(_BF),
        })
    res = run_bass_kernel_spmd(nc, in_maps, core_ids=list(range(NC_CORES)))
    out = np.empty((B, T, D), dtype=np.float32)
    for b in range(B):
        out[b] = res.results[2 * b]["y"] + res.results[2 * b + 1]["y"]
    return out


# revision 14
# speedup vs baseline: 3.2264x; 1.1906x over previous
"""LucidLinearAttention Trainium2 kernel (8-core SPMD), bf16 dataflow.

Sharding: batch b = core//2 (4 batches), head-group hg = core%2 (8 heads each).
Each core: qkv projection for its heads, chunked linear attention with a
hybrid block-causal formulation (256-col attention blocks, 64-wide buckets,
exclusive block carry C in f32), partial output projection. Host sums the two
head-group partials per batch.

Head-pair (h2) stacking keeps every projection / out-projection matmul at
M=K=128. S^T is computed only over the causally visible column range; the
bucket mask is realized by copying rect+wedge regions into pre-zeroed SBUF
tiles. All matmul operands are bf16 (predicted rel err ~6e-3 vs 2e-2 gate);
PSUM accumulation stays f32.

Scheduling: the PE stream is software-pipelined at two levels. The
out-projection of attention block a-1 and the Q/K/V projection + K^T
transposes of projection block N+1 are emitted as "filler" between the OUT
iterations of the current attention block, so the PE never idles on the
DVE/ACT normalization chain (recip -> dinv broadcast -> xot multiply).
"""
import sys
import numpy as np

for p in ("/opt/trn_rl_repo", "/root/.axon_site/_ro/trn_rl_repo"):
    if p not in sys.path:
        sys.path.insert(0, p)

import ml_dtypes
import concourse.mybir as mybir
import concourse.tile as tile
from concourse import bacc
from concourse.bass_utils import run_bass_kernel_spmd
from concourse.masks import make_identity

F32 = mybir.dt.float32
F32R = mybir.dt.float32r
BF16 = mybir.dt.bfloat16
EXP = mybir.ActivationFunctionType.Exp

B, T, D = 4, 4096, 1024
NH, HD, BUCKET = 16, 64, 64
HPC = 8                 # heads per core
GD = HPC * HD           # 512
NPB = 8                 # projection blocks
PBT = T // NPB          # 512 cols
NAB = 16                # attention blocks
ABT = T // NAB          # 256 cols
NC_CORES = 8

_CACHE = {}
_BF = ml_dtypes.bfloat16


def _build():
    nc = bacc.Bacc("TRN2", target_bir_lowering=False, debug=False,
                   num_devices=NC_CORES)
    xT = nc.dram_tensor("xT", [D, T], BF16, kind="ExternalInput").ap()
    wqT = nc.dram_tensor("wqT", [D, GD], BF16, kind="ExternalInput").ap()
    wkT = nc.dram_tensor("wkT", [D, GD], BF16, kind="ExternalInput").ap()
    wvT = nc.dram_tensor("wvT", [D, GD], BF16, kind="ExternalInput").ap()
    woT = nc.dram_tensor("woT", [GD, D], BF16, kind="ExternalInput").ap()
    y = nc.dram_tensor("y", [T, D], F32, kind="ExternalOutput").ap()

    with tile.TileContext(nc) as tc:
        with nc.allow_low_precision(reason="bf16 matmul dataflow by design"), \
             tc.tile_pool(name="w", bufs=1) as wp, \
             tc.tile_pool(name="per", bufs=1) as pp, \
             tc.tile_pool(name="sb", bufs=1) as sbp, \
             tc.tile_pool(name="ps", bufs=1, space="PSUM") as ps:

            # ---- resident weights (direct bf16 DMA) ---------------------
            wq_sb = [wp.tile([128, GD], BF16, tag=f"wq{dc}", name=f"wq{dc}") for dc in range(8)]
            wk_sb = [wp.tile([128, GD], BF16, tag=f"wk{dc}", name=f"wk{dc}") for dc in range(8)]
            wv_sb = [wp.tile([128, GD], BF16, tag=f"wv{dc}", name=f"wv{dc}") for dc in range(8)]
            wo_sb = [wp.tile([128, D], BF16, tag=f"wo{h2}", name=f"wo{h2}") for h2 in range(4)]

            xtiles = {}

            def load_x(pb0):
                xs = [sbp.tile([128, 2 * PBT], BF16, tag=f"xsb{dc}", name=f"xsb{dc}", bufs=2)
                      for dc in range(8)]
                for dc in range(8):
                    nc.sync.dma_start(
                        xs[dc][:], xT[128 * dc:128 * (dc + 1), PBT * pb0:PBT * (pb0 + 2)])
                xtiles[pb0] = xs
                xtiles[pb0 + 1] = xs

            for dc in range(8):
                nc.sync.dma_start(wk_sb[dc][:], wkT[128 * dc:128 * (dc + 1), :])
            load_x(0)
            for dc in range(8):
                nc.sync.dma_start(wq_sb[dc][:], wqT[128 * dc:128 * (dc + 1), :])
            for dc in range(8):
                nc.sync.dma_start(wv_sb[dc][:], wvT[128 * dc:128 * (dc + 1), :])
            for h2 in range(4):
                nc.sync.dma_start(wo_sb[h2][:], woT[128 * h2:128 * (h2 + 1), :])

            # ---- persistent state --------------------------------------
            ident_f = pp.tile([128, 128], F32, tag="ident_f")
            make_identity(nc, ident_f[:])
            ident_b = pp.tile([128, 128], BF16, tag="ident_b")
            nc.vector.tensor_copy(ident_b[:], ident_f[:])
            # dinv broadcast weights: top half / bottom half of a head pair
            bv_f = pp.tile([1, 256], F32, tag="bv_f")
            nc.vector.memset(bv_f[:], 0.0)
            nc.vector.memset(bv_f[0:1, 0:64], 1.0)
            nc.vector.memset(bv_f[0:1, 192:256], 1.0)
            bvt = pp.tile([1, 128], F32R, tag="bvt")
            nc.vector.tensor_copy(bvt[:], bv_f[0:1, 0:128])
            bvb = pp.tile([1, 128], F32R, tag="bvb")
            nc.vector.tensor_copy(bvb[:], bv_f[0:1, 128:256])
            # caug[h]: [128, 66] bf16; even h data in rows 0:64, odd in 64:128,
            # other half stays zero (K=128 padding for the inter matmul).
            caug = [pp.tile([128, 66], BF16, tag=f"caug{h}", name=f"caug{h}") for h in range(HPC)]
            for h in range(HPC):
                nc.vector.memset(caug[h][:], 0.0)
            caug_f32 = [pp.tile([64, 66], F32, tag=f"caugf{h}", name=f"caugf{h}") for h in range(HPC)]
            for h in range(HPC):
                nc.vector.memset(caug_f32[h][:], 0.0)
            # vaug[par][c]: [128, 8*66] bf16; per head slot: [v(64) | 1 | 0]
            vaug = [[pp.tile([128, HPC * 66], BF16, tag=f"vaug{par}_{c}", name=f"vaug{par}_{c}")
                     for c in range(4)] for par in range(2)]
            for par in range(2):
                for c in range(4):
                    nc.vector.memset(vaug[par][c][:], 0.0)
                    for h in range(HPC):
                        nc.vector.memset(vaug[par][c][:, 66 * h + 64:66 * h + 65], 1.0)
            # ssb[h][par]: [128, 320] bf16 masked S^T; cols 0:256 chunk0
            # (queries 0:256), cols 256:320 chunk1 wedge (queries 192:256).
            # Zero strips preset once.
            ssb = [[pp.tile([128, 320], BF16, tag=f"ssb{h}_{s}", name=f"ssb{h}_{s}")
                    for s in range(2)] for h in range(HPC)]
            for h in range(HPC):
                for s in range(2):
                    nc.vector.memset(ssb[h][s][:], 0.0)

            # ---- projection-work fillers -------------------------------
            def proj_filler(kind, idx, pb2, store):
                par2 = pb2 % 2
                xoff = PBT * par2

                def emit():
                    xs = xtiles[pb2]
                    if kind == "q":
                        pq = ps.tile([128, PBT], F32, tag="proj", name="pq", bufs=2)
                        for dc in range(8):
                            nc.tensor.matmul(
                                pq[:], wq_sb[dc][:, 128 * idx:128 * (idx + 1)],
                                xs[dc][:, xoff:xoff + PBT],
                                start=(dc == 0), stop=(dc == 7))
                        qt = sbp.tile([128, PBT], BF16, tag=f"qtu{idx}",
                                      name=f"qtu{idx}", bufs=2)
                        nc.scalar.activation(qt[:], pq[:], EXP)
                        store["qtu2"][idx] = qt
                    elif kind == "k":
                        pk = ps.tile([128, GD], F32, tag="proj", name="pk", bufs=2)
                        for dc in range(8):
                            nc.tensor.matmul(
                                pk[:], xs[dc][:, xoff + 128 * idx:xoff + 128 * (idx + 1)],
                                wk_sb[dc][:],
                                start=(dc == 0), stop=(dc == 7))
                        kt_ = sbp.tile([128, GD], BF16, tag=f"ksb{idx}",
                                       name=f"ksb{idx}", bufs=2)
                        nc.scalar.activation(kt_[:], pk[:], EXP)
                        store["ksb"][idx] = kt_
                    elif kind == "v":
                        pv = ps.tile([128, GD], F32, tag="proj", name="pv", bufs=2)
                        for dc in range(8):
                            nc.tensor.matmul(
                                pv[:], xs[dc][:, xoff + 128 * idx:xoff + 128 * (idx + 1)],
                                wv_sb[dc][:],
                                start=(dc == 0), stop=(dc == 7))
                        vv = vaug[par2][idx][:].rearrange("p (h c) -> p h c", c=66)
                        pvv = pv[:].rearrange("p (h c) -> p h c", c=64)
                        nc.vector.tensor_copy(vv[:, :, 0:64], pvv[:, :, :])
                    else:  # "t": K^T transpose for head pair idx
                        ktp = ps.tile([128, PBT], BF16, tag="proj", name="ktp", bufs=2)
                        for c in range(4):
                            nc.tensor.transpose(
                                ktp[:, 128 * c:128 * (c + 1)],
                                store["ksb"][c][:, 128 * idx:128 * (idx + 1)], ident_b[:])
                        kt = sbp.tile([128, PBT], BF16, tag=f"kt2{idx}",
                                      name=f"kt2{idx}", bufs=2)
                        nc.vector.tensor_copy(kt[:], ktp[:])
                        store["kt2"][idx] = kt
                return emit

            ORDER = ([("k", c) for c in range(4)]
                     + [("q", 0), ("v", 0), ("q", 1), ("v", 1),
                        ("q", 2), ("v", 2), ("q", 3), ("v", 3)]
                     + [("t", h2) for h2 in range(4)])

            def make_fillers(pb2, store):
                return [proj_filler(kind, idx, pb2, store) for kind, idx in ORDER]

            # prologue: block 0 projections emitted up front
            cur = {"qtu2": [None] * 4, "ksb": [None] * 4, "kt2": [None] * 4}
            for fi, f in enumerate(make_fillers(0, cur)):
                f()
                if fi == 3:
                    load_x(2)

            pending = []  # out-projection work deferred one attn block

            # ---- main loop over attention blocks -----------------------
            for pb in range(NPB):
                par = pb % 2
                if pb >= 2 and par == 0 and pb + 2 < NPB:
                    load_x(pb + 2)

                nxt = {"qtu2": [None] * 4, "ksb": [None] * 4, "kt2": [None] * 4}
                fillers = make_fillers(pb + 1, nxt) if pb + 1 < NPB else []
                qtu2, ksb, kt2 = cur["qtu2"], cur["ksb"], cur["kt2"]

                for ab in range(2):
                    a = 2 * pb + ab
                    qoff = ABT * ab
                    spar = a % 2
                    fq = fillers[8 * ab:8 * ab + 8]
                    xot2 = [sbp.tile([128, ABT], BF16, tag=f"xot{h2}",
                                     name=f"xot{h2}", bufs=2) for h2 in range(4)]

                    # S^T (visible range only) + masked rect/wedge copies
                    for h in range(HPC):
                        h2, hb = h // 2, (h % 2) * 64
                        pst = ps.tile([128, ABT], F32, tag="spb", name="pst", bufs=3)
                        nc.tensor.matmul(
                            pst[:, 0:192],
                            kt2[h2][hb:hb + 64, qoff:qoff + 128],
                            qtu2[h2][hb:hb + 64, qoff + 64:qoff + 256],
                            start=True, stop=True)
                        nc.tensor.matmul(
                            pst[:, 192:256],
                            kt2[h2][hb:hb + 64, qoff + 128:qoff + 256],
                            qtu2[h2][hb:hb + 64, qoff + 192:qoff + 256],
                            start=True, stop=True)
                        sb_t = ssb[h][spar]
                        nc.vector.tensor_copy(sb_t[:, 128:256], pst[:, 64:192])
                        wsrc = pst[0:64, :].rearrange("p (a b) -> p a b", b=64)
                        wdst = sb_t[0:64, 64:320].rearrange("p (a b) -> p a b", b=64)
                        nc.scalar.copy(wdst[:, 0:4:3, :], wsrc[:, 0:4:3, :])

                    # OUT groups + normalization, with out-projection of the
                    # previous attn block and next-block projection work
                    # interleaved as PE filler.
                    opbs, dvps = [None] * HPC, [None] * HPC
                    ow = pending.pop() if pending else None
                    ysb_cur = [None]

                    def norm_pair(p):
                        opbE = opbs[2 * p]
                        nc.tensor.matmul(opbE[0:128, 256:512], bvt[:, :],
                                         dvps[2 * p][:], start=True, stop=False)
                        nc.tensor.matmul(opbE[0:128, 256:512], bvb[:, :],
                                         dvps[2 * p + 1][:], start=False, stop=True)
                        xu = sbp.tile([128, ABT], BF16, tag="xotu", name="xotu", bufs=3)
                        nc.scalar.copy(xu[0:64, :], opbE[0:64, 0:256])
                        nc.scalar.copy(xu[64:128, :], opbs[2 * p + 1][0:64, 0:256])
                        nc.vector.tensor_mul(
                            xot2[p][:], opbE[:, 256:512], xu[:])

                    def emit_py_group(g):
                        xot2_p, a_p = ow
                        tch, fc = g // 2, g % 2
                        if fc == 0:
                            ysb_cur[0] = sbp.tile([128, D], F32, tag="ysb",
                                                  name="ysb", bufs=3)
                        py = ps.tile([128, GD], F32, tag="proj", name="py", bufs=2)
                        for h2p in range(4):
                            nc.tensor.matmul(
                                py[:],
                                xot2_p[h2p][:, 128 * tch:128 * (tch + 1)],
                                wo_sb[h2p][:, GD * fc:GD * (fc + 1)],
                                start=(h2p == 0), stop=(h2p == 3))
                        nc.scalar.copy(ysb_cur[0][:, GD * fc:GD * (fc + 1)], py[:])
                        if fc == 1:
                            r0 = ABT * a_p + 128 * tch
                            nc.sync.dma_start(y[r0:r0 + 128, :], ysb_cur[0][:])

                    for h in range(HPC):
                        h2, hb = h // 2, (h % 2) * 64
                        opb = ps.tile([128, 512], F32, tag="opb", name="opb", bufs=3)
                        opbs[h] = opb
                        nc.tensor.matmul(opb[0:66, 0:256], caug[h][:],
                                         qtu2[h2][:, qoff:qoff + 256],
                                         start=True, stop=False)
                        nc.tensor.matmul(opb[0:66, 64:256],
                                         vaug[par][2 * ab][:, 66 * h:66 * h + 66],
                                         ssb[h][spar][:, 64:256],
                                         start=False, stop=False)
                        nc.tensor.matmul(opb[0:66, 192:256],
                                         vaug[par][2 * ab + 1][:, 66 * h:66 * h + 66],
                                         ssb[h][spar][:, 256:320],
                                         start=False, stop=True)
                        # C / kcum update for this head (inter read is done)
                        pc = ps.tile([64, 66], F32, tag="spb", name="pc", bufs=3)
                        for ci in range(2):
                            c = 2 * ab + ci
                            nc.tensor.matmul(
                                pc[:], ksb[c][:, 64 * h:64 * (h + 1)],
                                vaug[par][c][:, 66 * h:66 * h + 66],
                                start=(ci == 0), stop=(ci == 1))
                        nc.vector.tensor_add(caug_f32[h][:], caug_f32[h][:], pc[:])
                        nc.gpsimd.tensor_copy(caug[h][hb:hb + 64, :], caug_f32[h][:])
                        dv = sbp.tile([1, ABT], F32R, tag="dv", name="dv", bufs=6)
                        dvps[h] = dv
                        if a == 0:
                            nc.vector.tensor_scalar_max(dv[:], opb[64:65, 0:256], 1e-30)
                            nc.vector.reciprocal(dv[:], dv[:])
                        else:
                            nc.vector.reciprocal(dv[:], opb[64:65, 0:256])
                        if h < len(fq):
                            fq[h]()
                        if ow is not None and h % 2 == 1:
                            emit_py_group(h // 2)
                        if h % 2 == 0 and h >= 2:
                            norm_pair((h - 2) // 2)
                    norm_pair(3)

                    pending.append((xot2, a))

                cur = nxt

            # epilogue: last attention block's out-projection
            ow = pending.pop()
            ysb_cur = [None]
            for g in range(4):
                emit_py_group(g)

    nc.compile()
    return nc


def _get_nc():
    if "nc" not in _CACHE:
        _CACHE["nc"] = _build()
    return _CACHE["nc"]


def kernel(x, W_qkv, W_out):
    x = np.asarray(x, dtype=np.float32)
    W_qkv = np.asarray(W_qkv, dtype=np.float32)
    W_out = np.asarray(W_out, dtype=np.float32)
    nc = _get_nc()

    xTs = [np.ascontiguousarray(x[b].T).astype(_BF) for b in range(B)]
    in_maps = []
    for c in range(NC_CORES):
        b, hg = c // 2, c % 2
        s = slice(hg * GD, (hg + 1) * GD)
        in_maps.append({
            "xT": xTs[b],
            "wqT": np.ascontiguousarray(W_qkv[0 * D:1 * D][s].T).astype(_BF),
            "wkT": np.ascontiguousarray(W_qkv[1 * D:2 * D][s].T).astype(_BF),
            "wvT": np.ascontiguousarray(W_qkv[2 * D:3 * D][s].T).astype(_BF),
            "woT": np.ascontiguousarray(W_out[:, s].T).astype(_BF),
        })
    res = run_bass_kernel_spmd(nc, in_maps, core_ids=list(range(NC_CORES)))
    out = np.empty((B, T, D), dtype=np.float32)
    for b in range(B):
        out[b] = res.results[2 * b]["y"] + res.results[2 * b + 1]["y"]
    return out
